# revision 3
# baseline (speedup 1.0000x reference)
import sys
import numpy as np

sys.path.insert(0, "/opt/trn_rl_repo")
from concourse import bass, mybir  # noqa: E402
from concourse.bass_utils import run_bass_kernel_spmd  # noqa: E402

T = 256
OBS = 48
LAT = 8
N = 2048          # LAT * T
NT = 14           # Chebyshev terms T_1..T_NT computed on device
NS = 16           # 128-row slabs of the 2048 x 2048 operand
COLS = 257        # 256 identity columns (per-core block) + 1 column carrying r
F32 = mybir.dt.float32

_nc_cache = None
TRACE = False
LAST_EXEC_NS = 0


def _build_nc():
    nc = bass.Bass(target_bir_lowering=False)
    X2 = nc.declare_dram_parameter("X2", [N, N], F32, isOutput=False)
    P0 = nc.declare_dram_parameter("P0", [N, COLS], F32, isOutput=False)
    TB = nc.declare_dram_parameter("TB", [NT, N, COLS], F32, isOutput=True)

    with (
        nc.semaphore("dmain_sem") as dmain,
        nc.semaphore("dmaout_sem") as dmaout,
        nc.semaphore("mm_sem") as mm_sem,
        nc.semaphore("vec_sem") as vec_sem,
        nc.sbuf_tensor("xsb", [128, NS * N], F32) as xsb,
        nc.sbuf_tensor("bufA", [128, NS * COLS], F32) as bufA,
        nc.sbuf_tensor("bufB", [128, NS * COLS], F32) as bufB,
        nc.sbuf_tensor("bufC", [128, NS * COLS], F32) as bufC,
        nc.psum_tensor("ps0", [128, COLS], F32) as ps0,
        nc.psum_tensor("ps1", [128, COLS], F32) as ps1,
        nc.psum_tensor("ps2", [128, COLS], F32) as ps2,
        nc.psum_tensor("ps3", [128, COLS], F32) as ps3,
    ):
        bufs = [bufA, bufB, bufC]
        psums = [ps0, ps1, ps2, ps3]

        with nc.Block() as block:

            @block.gpsimd
            def _(g):
                for s in range(NS):
                    g.dma_start(
                        out=xsb[:, s * N:(s + 1) * N],
                        in_=X2[s * 128:(s + 1) * 128, :],
                    ).then_inc(dmain, 16)
                for s in range(NS):
                    g.dma_start(
                        out=bufA[:, s * COLS:(s + 1) * COLS],
                        in_=P0[s * 128:(s + 1) * 128, :],
                    ).then_inc(dmain, 16)

            @block.tensor
            def _(te):
                te.wait_ge(dmain, 32 * 16)
                G = 0
                for t in range(1, NT + 1):
                    rbuf = bufs[(t - 1) % 3]
                    for m in range(NS):
                        w = max(G - 3, (t - 1) * 16)
                        if w > 0:
                            te.wait_ge(vec_sem, w)
                        ps = psums[G % 4]
                        for s in range(NS):
                            te.matmul(
                                ps[:, :],
                                xsb[:, s * N + m * 128: s * N + m * 128 + 128],
                                rbuf[:, s * COLS:(s + 1) * COLS],
                                start=(s == 0),
                                stop=(s == NS - 1),
                            ).then_inc(mm_sem)
                        G += 1

            @block.vector
            def _(v):
                G = 0
                for t in range(1, NT + 1):
                    wbuf = bufs[t % 3]
                    pbuf = bufs[(t - 2) % 3]
                    for m in range(NS):
                        v.wait_ge(mm_sem, 16 * (G + 1))
                        if t >= 4:
                            v.wait_ge(dmaout, 16 * ((t - 4) * 16 + m + 1))
                        ps = psums[G % 4]
                        dst = wbuf[:, m * COLS:(m + 1) * COLS]
                        if t == 1:
                            v.tensor_scalar_mul(dst, ps[:, :], 0.5).then_inc(vec_sem)
                        else:
                            v.tensor_sub(
                                dst, ps[:, :], pbuf[:, m * COLS:(m + 1) * COLS]
                            ).then_inc(vec_sem)
                        G += 1

            @block.sync
            def _(sy):
                G = 0
                for t in range(1, NT + 1):
                    wbuf = bufs[t % 3]
                    for m in range(NS):
                        sy.wait_ge(vec_sem, G + 1)
                        sy.dma_start(
                            out=TB[t - 1, m * 128:(m + 1) * 128, :],
                            in_=wbuf[:, m * COLS:(m + 1) * COLS],
                        ).then_inc(dmaout, 16)
                        G += 1

    return nc


def kernel(y, W, K_blocks, a, b):
    global _nc_cache
    y64 = np.asarray(y, np.float64)
    W64 = np.asarray(W, np.float64)
    K64 = np.asarray(K_blocks, np.float64)
    a64 = np.asarray(a, np.float64)
    b64 = np.asarray(b, np.float64)

    # host prep: small dense algebra (exact collapse of kron structure)
    M8 = (W64 * a64[:, None]).T @ W64                       # [LAT, LAT]
    Kinv = np.stack([np.linalg.inv(k) for k in K64])        # [LAT, T, T]
    logdetK = float(sum(np.linalg.slogdet(k)[1] for k in K64))
    Sig = np.zeros((N, N))
    for i in range(LAT):
        ii = slice(i * T, (i + 1) * T)
        for j in range(LAT):
            if i == j:
                Sig[ii, ii] = Kinv[i] + 2.0 * M8[i, i] * np.eye(T)
            else:
                Sig[ii, j * T:(j + 1) * T] = 2.0 * M8[i, j] * np.eye(T)
    r = (((y64 - b64[None, :]) @ W64).T).reshape(-1)        # [N]

    # spectral interval for the Chebyshev expansion
    ev = np.linalg.eigvalsh(Sig)
    LO, HI = 0.95 * ev[0], 1.05 * ev[-1]

    X2 = (2.0 * (2.0 * Sig - (LO + HI) * np.eye(N)) / (HI - LO)).astype(np.float32)
    r32 = r.astype(np.float32)

    if _nc_cache is None:
        _nc_cache = _build_nc()
    nc = _nc_cache

    in_maps = []
    for j in range(8):
        P0j = np.zeros((N, COLS), np.float32)
        for q in range(256):
            P0j[256 * j + q, q] = 1.0
        P0j[:, 256] = r32
        in_maps.append({"X2": X2, "P0": P0j})

    rr = run_bass_kernel_spmd(nc, in_maps, list(range(8)), trace=TRACE)
    if TRACE:
        global LAST_EXEC_NS
        LAST_EXEC_NS = rr.exec_time_ns or 0
        if rr.instructions_and_trace:
            print("trace:", rr.instructions_and_trace[1])
    res = rr.results
    TBs = [np.asarray(res[j]["TB"]) for j in range(8)]

    tr = [float(N)]
    q = [float(r32 @ r32)]
    idx = np.arange(256)
    for k in range(NT):
        tr.append(float(sum(TBs[j][k][256 * j + idx, idx].sum() for j in range(8))))
        q.append(float(r32 @ TBs[0][k][:, 256]))

    g = np.linspace(LO, HI, 4000)
    cl = np.polynomial.chebyshev.Chebyshev.fit(g, np.log(g), deg=NT, domain=(LO, HI)).coef
    ci = np.polynomial.chebyshev.Chebyshev.fit(g, 1.0 / g, deg=NT, domain=(LO, HI)).coef
    logdetSig = sum(cl[k] * tr[k] for k in range(NT + 1))
    rAr = sum(ci[k] * q[k] for k in range(NT + 1))
    out = 0.5 * logdetSig + 0.5 * rAr + 0.5 * logdetK
    return np.float32(out)


# revision 7
# speedup vs baseline: 1.2326x; 1.2326x over previous
import sys
import numpy as np

sys.path.insert(0, "/opt/trn_rl_repo")
from concourse import bass, mybir  # noqa: E402
from concourse.bass_utils import run_bass_kernel_spmd  # noqa: E402

T = 256
OBS = 48
LAT = 8
N = 2048          # LAT * T
NT = 14           # Chebyshev terms T_1..T_NT computed on device
NS = 16           # 128-row slabs of the 2048 x 2048 operand
COLS = 257        # 256 identity columns (per-core block) + 1 column carrying r
F32 = mybir.dt.float32

_nc_cache = None
TRACE = False
LAST_EXEC_NS = 0


def _build_nc():
    nc = bass.Bass(target_bir_lowering=False)
    X2 = nc.declare_dram_parameter("X2", [N, N], F32, isOutput=False)
    P0 = nc.declare_dram_parameter("P0", [N, COLS], F32, isOutput=False)
    TB = nc.declare_dram_parameter("TB", [NT, N, COLS], F32, isOutput=True)

    with (
        nc.semaphore("dmain_sem") as dmain,
        nc.semaphore("dmaout_sem") as dmaout,
        nc.semaphore("mm_sem") as mm_sem,
        nc.semaphore("vec_sem") as vec_sem,
        nc.sbuf_tensor("xsb", [128, NS * N], F32) as xsb,
        nc.sbuf_tensor("bufA", [128, NS * COLS], F32) as bufA,
        nc.sbuf_tensor("bufB", [128, NS * COLS], F32) as bufB,
        nc.sbuf_tensor("bufC", [128, NS * COLS], F32) as bufC,
        nc.psum_tensor("ps0", [128, COLS], F32) as ps0,
        nc.psum_tensor("ps1", [128, COLS], F32) as ps1,
        nc.psum_tensor("ps2", [128, COLS], F32) as ps2,
        nc.psum_tensor("ps3", [128, COLS], F32) as ps3,
    ):
        bufs = [bufA, bufB, bufC]
        psums = [ps0, ps1, ps2, ps3]

        with nc.Block() as block:

            @block.gpsimd
            def _(g):
                for s in range(NS):
                    g.dma_start(
                        out=xsb[:, s * N:(s + 1) * N],
                        in_=X2[s * 128:(s + 1) * 128, :],
                    ).then_inc(dmain, 16)
                for s in range(NS):
                    g.dma_start(
                        out=bufA[:, s * COLS:(s + 1) * COLS],
                        in_=P0[s * 128:(s + 1) * 128, :],
                    ).then_inc(dmain, 16)

            @block.tensor
            def _(te):
                te.wait_ge(dmain, 32 * 16)
                G = 0
                for t in range(1, NT + 1):
                    rbuf = bufs[(t - 1) % 3]
                    for m in range(NS):
                        w = max(G - 3, (t - 1) * 16)
                        if w > 0:
                            te.wait_ge(vec_sem, w)
                        ps = psums[G % 4]
                        for s in range(NS):
                            te.matmul(
                                ps[:, :],
                                xsb[:, s * N + m * 128: s * N + m * 128 + 128],
                                rbuf[:, s * COLS:(s + 1) * COLS],
                                start=(s == 0),
                                stop=(s == NS - 1),
                            ).then_inc(mm_sem)
                        G += 1

            @block.vector
            def _(v):
                G = 0
                for t in range(1, NT + 1):
                    wbuf = bufs[t % 3]
                    pbuf = bufs[(t - 2) % 3]
                    for m in range(NS):
                        v.wait_ge(mm_sem, 16 * (G + 1))
                        if t >= 4:
                            v.wait_ge(dmaout, 16 * ((t - 4) * 16 + m + 1))
                        ps = psums[G % 4]
                        dst = wbuf[:, m * COLS:(m + 1) * COLS]
                        if t == 1:
                            v.tensor_scalar_mul(dst, ps[:, :], 0.5).then_inc(vec_sem)
                        else:
                            v.tensor_sub(
                                dst, ps[:, :], pbuf[:, m * COLS:(m + 1) * COLS]
                            ).then_inc(vec_sem)
                        G += 1

            @block.sync
            def _(sy):
                G = 0
                for t in range(1, NT + 1):
                    wbuf = bufs[t % 3]
                    for m in range(NS):
                        sy.wait_ge(vec_sem, G + 1)
                        sy.dma_start(
                            out=TB[t - 1, m * 128:(m + 1) * 128, :],
                            in_=wbuf[:, m * COLS:(m + 1) * COLS],
                        ).then_inc(dmaout, 16)
                        G += 1

    return nc


def kernel(y, W, K_blocks, a, b):
    global _nc_cache
    y64 = np.asarray(y, np.float64)
    W64 = np.asarray(W, np.float64)
    K64 = np.asarray(K_blocks, np.float64)
    a64 = np.asarray(a, np.float64)
    b64 = np.asarray(b, np.float64)

    # host prep: small dense algebra (exact collapse of kron structure)
    M8 = (W64 * a64[:, None]).T @ W64                       # [LAT, LAT]
    Kinv = np.stack([np.linalg.inv(k) for k in K64])        # [LAT, T, T]
    logdetK = float(sum(np.linalg.slogdet(k)[1] for k in K64))
    Sig = np.zeros((N, N))
    for i in range(LAT):
        ii = slice(i * T, (i + 1) * T)
        for j in range(LAT):
            if i == j:
                Sig[ii, ii] = Kinv[i] + 2.0 * M8[i, i] * np.eye(T)
            else:
                Sig[ii, j * T:(j + 1) * T] = 2.0 * M8[i, j] * np.eye(T)
    r = (((y64 - b64[None, :]) @ W64).T).reshape(-1)        # [N]

    # spectral interval for the Chebyshev expansion
    ev = np.linalg.eigvalsh(Sig)
    LO, HI = 0.95 * ev[0], 1.05 * ev[-1]

    X2 = (2.0 * (2.0 * Sig - (LO + HI) * np.eye(N)) / (HI - LO)).astype(np.float32)
    r32 = r.astype(np.float32)

    if _nc_cache is None:
        _nc_cache = _build_nc()
    nc = _nc_cache

    in_maps = []
    for j in range(8):
        P0j = np.zeros((N, COLS), np.float32)
        for q in range(256):
            P0j[256 * j + q, q] = 1.0
        P0j[:, 256] = r32
        in_maps.append({"X2": X2, "P0": P0j})

    rr = run_bass_kernel_spmd(nc, in_maps, list(range(8)), trace=TRACE)
    if TRACE:
        global LAST_EXEC_NS
        LAST_EXEC_NS = rr.exec_time_ns or 0
        if rr.instructions_and_trace:
            print("trace:", rr.instructions_and_trace[1])
    res = rr.results
    TBs = [np.asarray(res[j]["TB"]) for j in range(8)]

    tr = [float(N)]
    q = [float(r32 @ r32)]
    idx = np.arange(256)
    for k in range(NT):
        tr.append(float(sum(TBs[j][k][256 * j + idx, idx].sum() for j in range(8))))
        q.append(float(r32 @ TBs[0][k][:, 256]))

    g = np.linspace(LO, HI, 4000)
    cl = np.polynomial.chebyshev.Chebyshev.fit(g, np.log(g), deg=NT, domain=(LO, HI)).coef
    ci = np.polynomial.chebyshev.Chebyshev.fit(g, 1.0 / g, deg=NT, domain=(LO, HI)).coef
    logdetSig = sum(cl[k] * tr[k] for k in range(NT + 1))
    rAr = sum(ci[k] * q[k] for k in range(NT + 1))
    out = 0.5 * logdetSig + 0.5 * rAr + 0.5 * logdetK
    return np.float32(out)
